# revision 1
# baseline (speedup 1.0000x reference)
"""MoE FFN (top-2 of 8 experts) Trainium2 kernel.

Strategy (expert-parallel across 8 NeuronCores):
  - Host computes the (tiny) router: logits = x@Wg, softmax, top-2,
    renormalized combine weights.  Tokens are gathered per expert on the
    host ("all-to-all dispatch" done at sharding time), transposed to
    [H, C] so both FFN GEMMs run with natural weight layouts on device.
  - Core e runs the FFN for expert e over its C_pad gathered tokens,
    F-quarter by F-quarter (quarter weights stream through SBUF,
    double-buffered; chunks of <=512 tokens bound PSUM/SBUF usage):
        hT = gelu_tanh(W1.T-tiles @ xT)        # [Fq, C] per quarter
        Y_fb = hT-tiles.T @ W2_fb              # [C, H] partial per quarter
    Partials land in per-quarter DRAM regions; the host sums them
    (cheaper than device-side DRAM read-back accumulation).
  - Host applies combine weights + b2 and scatter-adds back ("combine").

  All matmuls use float32r (full-rate fp32 tensor-engine mode, fp32
  storage, fp32 PSUM accumulation).

The kernel is compiled once per (C_pad, chunk-structure, biases-zero)
configuration and cached in-process.
"""

import os
import sys
import numpy as np

for _p in ("/opt/trn_rl_repo", "/root/.axon_site/_ro/trn_rl_repo"):
    if _p not in sys.path and os.path.isdir(_p):
        sys.path.append(_p)

import concourse.bacc as bacc  # noqa: E402
import concourse.tile as tile  # noqa: E402
from concourse import mybir  # noqa: E402
from concourse.bass_utils import run_bass_kernel_spmd  # noqa: E402

# Problem shapes (hardcoded per spec)
B, S, H, F, E = 4, 2048, 1024, 4096, 8
T = B * S
TOP_K = 2
N_CORES = 8
P = 128
KH = H // P          # 8  H-contraction subtiles
FT = F // P          # 32 f-tiles total
# F processed in blocks of f-tiles (weights resident per block, streamed
# double-buffered). Equal blocks of 8 measured best: smaller lead blocks
# shorten the head but cost more in GEMM2 accumulation-group overhead.
BLOCKS = (8, 8, 8, 8)
NBLK = len(BLOCKS)
MH = H // P          # 8  output H tiles

F32 = mybir.dt.float32
F32R = mybir.dt.float32r

_CACHE: dict = {}
LAST_RESULT = None  # BassKernelResults of the most recent run (for test.py)


def _chunks_for(c_pad: int) -> tuple:
    """Token chunks: 512s with an optional single 256 tail."""
    out = [512] * (c_pad // 512)
    if c_pad % 512:
        assert c_pad % 512 == 256
        out.append(256)
    return tuple(out)


def _build(c_pad: int, chunks: tuple, use_b1: bool, mm_dt, act_fn=None):
    nc = bacc.Bacc(
        "TRN2",
        target_bir_lowering=False,
        debug=False,
        enable_asserts=False,
        num_devices=N_CORES,
    )

    xd = nc.dram_tensor("xd", [P, KH, c_pad], mm_dt, kind="ExternalInput").ap()
    w1d = nc.dram_tensor("w1d", [P, FT, KH, P], mm_dt, kind="ExternalInput").ap()
    w2d = nc.dram_tensor("w2d", [P, FT, H], mm_dt, kind="ExternalInput").ap()
    if use_b1:
        b1d = nc.dram_tensor("b1d", [P, FT], F32, kind="ExternalInput").ap()
    # per-F-block partial outputs; host sums over the NBLK axis (cheaper than
    # device-side DRAM read-back accumulation, which stalls the PE)
    yd = nc.dram_tensor(
        "yd", [P, NBLK, c_pad // P, H], F32, kind="ExternalOutput"
    ).ap()

    gelu = act_fn or mybir.ActivationFunctionType.Gelu_apprx_tanh

    with tile.TileContext(nc) as tc:
        with (
            tc.tile_pool(name="w1p", bufs=2) as w1p,
            tc.tile_pool(name="w2p", bufs=2) as w2p,
            tc.tile_pool(name="xp", bufs=2) as xp,
            tc.tile_pool(name="hp", bufs=2) as hp,
            tc.tile_pool(name="op", bufs=6) as op,
            tc.tile_pool(name="bp", bufs=1) as bp,
            tc.tile_pool(name="ps1", bufs=3, space="PSUM") as ps1,
            tc.tile_pool(name="ps2", bufs=5, space="PSUM") as ps2,
        ):
            if use_b1:
                b1t = bp.tile([P, FT], F32)
                nc.sync.dma_start(b1t[:], b1d[:])

            fstart = 0
            for bi, fbn in enumerate(BLOCKS):
                # weights stream on the scalar HWDGE ring in ~1MB slices so
                # they never head-of-line-block the x/y traffic (sync ring)
                w1q = w1p.tile([P, fbn, KH, P], mm_dt, tag="w1q", name=f"w1q_{bi}")
                for f in range(fbn):
                    nc.scalar.dma_start(w1q[:, f], w1d[:, fstart + f])
                w2q = w2p.tile([P, fbn, H], mm_dt, tag="w2q", name=f"w2q_{bi}")
                for k2 in range(fbn):
                    nc.scalar.dma_start(w2q[:, k2], w2d[:, fstart + k2])

                coff = 0
                for ci, nt in enumerate(chunks):
                    xt = xp.tile([P, KH, nt], mm_dt, tag="xt")
                    if ci == 0:
                        # halves: GEMM1 k=0..3 can start on the first 1MB
                        nc.sync.dma_start(xt[:, :4], xd[:, :4, coff : coff + nt])
                        nc.sync.dma_start(xt[:, 4:], xd[:, 4:, coff : coff + nt])
                    else:
                        nc.sync.dma_start(xt[:], xd[:, :, coff : coff + nt])

                    # GEMM1: hT[f, :] = gelu(sum_k W1[k, f-tile].T @ xT[k, :])
                    hq = hp.tile([P, fbn, nt], mm_dt, tag="hq", name=f"hq_{bi}")
                    for f in range(fbn):
                        pt1 = ps1.tile([P, nt], F32, tag="pt1")
                        for k in range(KH):
                            nc.tensor.matmul(
                                pt1[:],
                                w1q[:, f, k, :],
                                xt[:, k, :],
                                start=(k == 0),
                                stop=(k == KH - 1),
                            )
                        bias = (
                            b1t[:, fstart + f : fstart + f + 1] if use_b1 else 0.0
                        )
                        nc.scalar.activation(hq[:, f, :], pt1[:], gelu, bias=bias)

                    # GEMM2 (partial over this F-block):
                    # Y[t-tile, hh] += sum_k2 hT[k2, t-tile].T @ W2[k2, hh]
                    for t in range(nt // P):
                        pts = [
                            ps2.tile([P, 512], F32, tag="pt2", name=f"pt2_{hh}")
                            for hh in range(2)
                        ]
                        for k2 in range(fbn):
                            for hh in range(2):
                                nc.tensor.matmul(
                                    pts[hh][:],
                                    hq[:, k2, t * P : (t + 1) * P],
                                    w2q[:, k2, hh * 512 : (hh + 1) * 512],
                                    start=(k2 == 0),
                                    stop=(k2 == fbn - 1),
                                )
                        trow = coff // P + t
                        for hh in range(2):
                            ot = op.tile([P, 512], F32, tag="ot")
                            dst = yd[:, bi, trow, hh * 512 : (hh + 1) * 512]
                            nc.vector.tensor_copy(ot[:], pts[hh][:])
                            nc.sync.dma_start(dst, ot[:])
                    coff += nt
                fstart += fbn

    nc.compile()
    return nc


def _route(x2d, Wg):
    """Replicates reference router: softmax -> top-2 -> renormalize."""
    logits = x2d @ Wg  # [T, E] fp32
    m = logits.max(axis=-1, keepdims=True)
    p = np.exp(logits - m, dtype=np.float32)
    p /= p.sum(axis=-1, keepdims=True)
    # jax.lax.top_k: values descending, ties broken by lower index.
    order = np.argsort(-p, axis=-1, kind="stable")
    top_i = order[:, :TOP_K]  # [T, 2]
    top_p = np.take_along_axis(p, top_i, axis=-1)
    top_p = top_p / top_p.sum(axis=-1, keepdims=True)
    return top_i, top_p


def kernel(x, Wg, W1, b1, W2, b2):
    global LAST_RESULT
    x = np.ascontiguousarray(np.asarray(x, dtype=np.float32))
    Wg = np.ascontiguousarray(np.asarray(Wg, dtype=np.float32))
    W1 = np.ascontiguousarray(np.asarray(W1, dtype=np.float32))
    b1 = np.ascontiguousarray(np.asarray(b1, dtype=np.float32))
    W2 = np.ascontiguousarray(np.asarray(W2, dtype=np.float32))
    b2 = np.ascontiguousarray(np.asarray(b2, dtype=np.float32))

    x2d = x.reshape(T, H)
    top_i, top_p = _route(x2d, Wg)

    rows = [None] * E
    gval = [None] * E
    for e in range(E):
        r, slot = np.nonzero(top_i == e)
        rows[e] = r
        gval[e] = top_p[r, slot]

    c_max = max(len(r) for r in rows)
    c_pad = max(512, ((c_max + 255) // 256) * 256)
    chunks = _chunks_for(c_pad)
    use_b1 = bool(np.any(b1))

    mm_dt = {
        "fp32r": F32R,
        "fp32": F32,
        "bf16": mybir.dt.bfloat16,
    }[os.environ.get("KERNEL_MMDT", "fp32r")]
    key = (c_pad, chunks, use_b1, str(mm_dt))
    if key not in _CACHE:
        _CACHE[key] = _build(c_pad, chunks, use_b1, mm_dt)
    nc = _CACHE[key]

    np_dt = mybir.dt.np(mm_dt)
    in_maps = []
    for e in range(E):
        ce = len(rows[e])
        xt = np.zeros((H, c_pad), np.float32)
        xt[:, :ce] = x2d[rows[e]].T
        m = {
            "xd": np.ascontiguousarray(
                xt.reshape(KH, P, c_pad).transpose(1, 0, 2).astype(np_dt)
            ),
            "w1d": np.ascontiguousarray(
                W1[e].reshape(KH, P, FT, P).transpose(1, 2, 0, 3).astype(np_dt)
            ),
            "w2d": np.ascontiguousarray(
                W2[e].reshape(FT, P, H).transpose(1, 0, 2).astype(np_dt)
            ),
        }
        if use_b1:
            m["b1d"] = np.ascontiguousarray(b1[e].reshape(FT, P).T)
        in_maps.append(m)

    trace = os.environ.get("KERNEL_TRACE", "") == "1"
    res = run_bass_kernel_spmd(
        nc,
        in_maps,
        core_ids=list(range(N_CORES)),
        trace=trace,
        trace_cores=[0] if trace else None,
    )
    LAST_RESULT = res

    out = np.zeros((T, H), np.float32)
    for e in range(E):
        ce = len(rows[e])
        yt = res.results[e]["yd"].sum(axis=1, dtype=np.float32)  # [P, c_pad//P, H]
        y = yt.transpose(1, 0, 2).reshape(c_pad, H)[:ce]
        out[rows[e]] += gval[e][:, None] * (y + b2[e][None, :])

    return out.reshape(B, S, H)



# revision 5
# speedup vs baseline: 1.0736x; 1.0736x over previous
"""MoE FFN (top-2 of 8 experts) Trainium2 kernel.

Strategy (expert-parallel across 8 NeuronCores):
  - Host computes the (tiny) router: logits = x@Wg, softmax, top-2,
    renormalized combine weights.  Tokens are gathered per expert on the
    host ("all-to-all dispatch" done at sharding time), transposed to
    [H, C] so both FFN GEMMs run with natural weight layouts on device.
  - Core e runs the FFN for expert e over its C_pad gathered tokens.
    Both expert weight matrices live fully resident in SBUF as bf16
    (64 KiB/partition each), streamed in fine f-slices at kernel start
    so compute begins after the first ~1 MiB lands.
  - Per 512-token chunk, two phases:
        phase 1:  hT[f,:] = gelu_tanh(sum_k W1[k,f].T @ xT[k,:])  all 32
                  f-tiles, kept resident in SBUF (bf16)
        phase 2:  yT[hs,:] += W2[f,hs].T @ hT[f,:]  accumulated over all
                  32 f-tiles directly in 8 PSUM banks (fp32) -> single
                  bf16 store per h-subtile, no partial-sum DRAM traffic.
  - Host applies combine weights + b2 and scatter-adds back ("combine").

  Matmuls run in bf16 (full-rate, FWL weight loads); PSUM accumulation
  is fp32.  rel-err vs the fp32 reference ~3e-3, well inside 2e-2.

The kernel is compiled once per (C_pad, chunk-structure, biases-zero)
configuration and cached in-process.
"""

import os
import sys
import numpy as np

for _p in ("/opt/trn_rl_repo", "/root/.axon_site/_ro/trn_rl_repo"):
    if _p not in sys.path and os.path.isdir(_p):
        sys.path.append(_p)

import concourse.bacc as bacc  # noqa: E402
import concourse.tile as tile  # noqa: E402
from concourse import mybir  # noqa: E402
from concourse.bass_utils import run_bass_kernel_spmd  # noqa: E402

# Problem shapes (hardcoded per spec)
B, S, H, F, E = 4, 2048, 1024, 4096, 8
T = B * S
TOP_K = 2
N_CORES = 8
P = 128
KH = H // P          # 8   H-contraction subtiles
FT = F // P          # 32  f-tiles
HS = H // P          # 8   output H subtiles
CH = 512             # token chunk (phase-1/2 granularity)
WG = 2               # f-tiles per weight-stream DMA slice

F32 = mybir.dt.float32
BF16 = mybir.dt.bfloat16

_CACHE: dict = {}
LAST_RESULT = None  # BassKernelResults of the most recent run (for test.py)


def _chunks_for(c_pad: int) -> tuple:
    out = [CH] * (c_pad // CH)
    if c_pad % CH:
        out.append(c_pad % CH)
    return tuple(out)


def _build(c_pad: int, chunks: tuple, use_b1: bool, mm_dt):
    nc = bacc.Bacc(
        "TRN2",
        target_bir_lowering=False,
        debug=False,
        enable_asserts=False,
        num_devices=N_CORES,
    )

    nch = len(chunks)
    # x packed chunk-contiguous: xd[p, ci, k, t] = x.T[k*128+p, ci*CH+t]
    xd = nc.dram_tensor("xd", [P, nch, KH, CH], mm_dt, kind="ExternalInput").ap()
    # w1d[p, f, k, q] = W1[k*128+p, f*128+q]
    w1d = nc.dram_tensor("w1d", [P, FT, KH, P], mm_dt, kind="ExternalInput").ap()
    # w2d[p, f, h]   = W2[f*128+p, h]
    w2d = nc.dram_tensor("w2d", [P, FT, H], mm_dt, kind="ExternalInput").ap()
    if use_b1:
        b1d = nc.dram_tensor("b1d", [P, FT], F32, kind="ExternalInput").ap()
    # yd[p, hs, t] = y[t, hs*128+p]  (bf16; host upcasts + combines)
    yd = nc.dram_tensor("yd", [P, HS, c_pad], BF16, kind="ExternalOutput").ap()

    gelu = mybir.ActivationFunctionType.Gelu_apprx_tanh

    with tile.TileContext(nc) as tc:
        with (
            tc.tile_pool(name="w1p", bufs=FT // WG) as w1p,
            tc.tile_pool(name="w2p", bufs=FT // WG) as w2p,
            tc.tile_pool(name="xp", bufs=2) as xp,
            tc.tile_pool(name="hp", bufs=FT + 1) as hp,
            tc.tile_pool(name="yp", bufs=10) as yp,
            tc.tile_pool(name="bp", bufs=1) as bp,
            tc.tile_pool(name="pp", bufs=1, space="PSUM") as pp,
        ):
            if use_b1:
                b1t = bp.tile([P, FT], F32)
                nc.scalar.dma_start(b1t[:], b1d[:])

            # first x chunk ahead of the weight stream (needed immediately)
            xts: dict = {}
            xts[0] = xp.tile([P, KH, CH], mm_dt, tag="xt", name="xt_0")
            nc.scalar.dma_start(xts[0][:, :, : chunks[0]], xd[:, 0, :, : chunks[0]])

            # resident weights, streamed in WG-f-tile slices (w1/w2
            # interleaved so phase 2 of chunk 0 never waits long)
            w1g = []
            w2g = []
            for g in range(FT // WG):
                t1 = w1p.tile([P, WG, KH, P], mm_dt, tag="w1g", name=f"w1g_{g}")
                nc.scalar.dma_start(t1[:], w1d[:, g * WG : (g + 1) * WG])
                w1g.append(t1)
                t2 = w2p.tile([P, WG, H], mm_dt, tag="w2g", name=f"w2g_{g}")
                nc.scalar.dma_start(t2[:], w2d[:, g * WG : (g + 1) * WG])
                w2g.append(t2)

            coff = 0
            for ci, nt in enumerate(chunks):
                xt = xts.pop(ci)
                # prefetch next chunk's tokens (behind the weight stream)
                if ci + 1 < nch:
                    nxt = chunks[ci + 1]
                    xts[ci + 1] = xp.tile(
                        [P, KH, CH], mm_dt, tag="xt", name=f"xt_{ci + 1}"
                    )
                    nc.scalar.dma_start(
                        xts[ci + 1][:, :, :nxt], xd[:, ci + 1, :, :nxt]
                    )

                # phase 1: hT = gelu(W1.T @ xT) for all 32 f-tiles
                hqs = []
                for f in range(FT):
                    g, j = divmod(f, WG)
                    pt = pp.tile([P, CH], F32, tag="pt", bufs=2)
                    for k in range(KH):
                        nc.tensor.matmul(
                            pt[:, :nt],
                            w1g[g][:, j, k, :],
                            xt[:, k, :nt],
                            start=(k == 0),
                            stop=(k == KH - 1),
                        )
                    hq = hp.tile([P, CH], mm_dt, tag="hq", name=f"hq_{ci}_{f}")
                    bias = b1t[:, f : f + 1] if use_b1 else 0.0
                    nc.scalar.activation(hq[:, :nt], pt[:, :nt], gelu, bias=bias)
                    hqs.append(hq)

                # phase 2: yT[hs] = sum_f W2[f,hs].T @ hT[f], accumulated in
                # PSUM over all 32 f-tiles.  Two passes over the h-subtiles
                # (0-5, then 6-7) keep peak PSUM usage at 6 accumulator banks
                # + 2 phase-1 banks; the second pass re-reads the SBUF-resident
                # hqs, costing nothing extra on the PE.
                def gemm2_pass(hs_list):
                    pys = [
                        pp.tile([P, CH], F32, tag="py", bufs=6,
                                name=f"py_{ci}_{hs}")
                        for hs in hs_list
                    ]
                    for f in range(FT):
                        g, j = divmod(f, WG)
                        for pi, hs in enumerate(hs_list):
                            nc.tensor.matmul(
                                pys[pi][:, :nt],
                                w2g[g][:, j, hs * P : (hs + 1) * P],
                                hqs[f][:, :nt],
                                start=(f == 0),
                                stop=(f == FT - 1),
                            )
                    # evacuate: PSUM fp32 -> SBUF bf16 -> DRAM
                    # (DVE even, ACT odd, in PE-demand order)
                    for pi, hs in enumerate(hs_list):
                        yt = yp.tile([P, CH], BF16, tag="yt")
                        if pi % 2 == 0:
                            nc.vector.tensor_copy(yt[:, :nt], pys[pi][:, :nt])
                        else:
                            nc.scalar.copy(yt[:, :nt], pys[pi][:, :nt])
                        nc.sync.dma_start(
                            yd[:, hs, coff : coff + nt], yt[:, :nt]
                        )

                gemm2_pass(list(range(6)))
                gemm2_pass([6, 7])
                coff += nt

    nc.compile()
    return nc


def _route(x2d, Wg):
    """Replicates reference router: softmax -> top-2 -> renormalize."""
    logits = x2d @ Wg  # [T, E] fp32
    m = logits.max(axis=-1, keepdims=True)
    p = np.exp(logits - m, dtype=np.float32)
    p /= p.sum(axis=-1, keepdims=True)
    # jax.lax.top_k: values descending, ties broken by lower index.
    order = np.argsort(-p, axis=-1, kind="stable")
    top_i = order[:, :TOP_K]  # [T, 2]
    top_p = np.take_along_axis(p, top_i, axis=-1)
    top_p = top_p / top_p.sum(axis=-1, keepdims=True)
    return top_i, top_p


def kernel(x, Wg, W1, b1, W2, b2):
    global LAST_RESULT
    x = np.ascontiguousarray(np.asarray(x, dtype=np.float32))
    Wg = np.ascontiguousarray(np.asarray(Wg, dtype=np.float32))
    W1 = np.ascontiguousarray(np.asarray(W1, dtype=np.float32))
    b1 = np.ascontiguousarray(np.asarray(b1, dtype=np.float32))
    W2 = np.ascontiguousarray(np.asarray(W2, dtype=np.float32))
    b2 = np.ascontiguousarray(np.asarray(b2, dtype=np.float32))

    x2d = x.reshape(T, H)
    top_i, top_p = _route(x2d, Wg)

    rows = [None] * E
    gval = [None] * E
    for e in range(E):
        r, slot = np.nonzero(top_i == e)
        rows[e] = r
        gval[e] = top_p[r, slot]

    c_pad = max(CH, max(len(r) for r in rows))
    chunks = _chunks_for(c_pad)
    use_b1 = bool(np.any(b1))

    mm_dt = {
        "bf16": BF16,
        "fp32r": mybir.dt.float32r,
        "fp32": F32,
    }[os.environ.get("KERNEL_MMDT", "bf16")]
    key = (c_pad, chunks, use_b1, str(mm_dt))
    if key not in _CACHE:
        _CACHE[key] = _build(c_pad, chunks, use_b1, mm_dt)
    nc = _CACHE[key]

    np_dt = mybir.dt.np(mm_dt)
    nch = len(chunks)
    in_maps = []
    for e in range(E):
        ce = len(rows[e])
        xt = np.zeros((H, c_pad), np.float32)
        xt[:, :ce] = x2d[rows[e]].T
        # [H, c_pad] -> [P, nch, KH, CH] chunk-contiguous
        xpk = np.zeros((P, nch, KH, CH), np_dt)
        xk = xt.reshape(KH, P, c_pad)
        coff = 0
        for ci, nt in enumerate(chunks):
            xpk[:, ci, :, :nt] = (
                xk[:, :, coff : coff + nt].transpose(1, 0, 2).astype(np_dt)
            )
            coff += nt
        m = {
            "xd": np.ascontiguousarray(xpk),
            "w1d": np.ascontiguousarray(
                W1[e].reshape(KH, P, FT, P).transpose(1, 2, 0, 3).astype(np_dt)
            ),
            "w2d": np.ascontiguousarray(
                W2[e].reshape(FT, P, H).transpose(1, 0, 2).astype(np_dt)
            ),
        }
        if use_b1:
            m["b1d"] = np.ascontiguousarray(b1[e].reshape(FT, P).T)
        in_maps.append(m)

    trace = os.environ.get("KERNEL_TRACE", "") == "1"
    res = run_bass_kernel_spmd(
        nc,
        in_maps,
        core_ids=list(range(N_CORES)),
        trace=trace,
        trace_cores=[0] if trace else None,
    )
    LAST_RESULT = res

    out = np.zeros((T, H), np.float32)
    for e in range(E):
        ce = len(rows[e])
        yt = res.results[e]["yd"]  # [P, HS, c_pad] bf16
        y = yt.transpose(2, 1, 0).reshape(c_pad, H)[:ce].astype(np.float32)
        out[rows[e]] += gval[e][:, None] * (y + b2[e][None, :])

    return out.reshape(B, S, H)


# revision 8
# speedup vs baseline: 1.1474x; 1.0687x over previous
"""MoE FFN (top-2 of 8 experts) Trainium2 kernel.

Strategy (expert-parallel across 8 NeuronCores):
  - Host computes the (tiny) router: logits = x@Wg, softmax, top-2,
    renormalized combine weights.  Tokens are gathered per expert on the
    host ("all-to-all dispatch" done at sharding time), transposed to
    [H, C] so both FFN GEMMs run with natural weight layouts on device.
  - Core e runs the FFN for expert e over its C_pad gathered tokens.
    Both expert weight matrices live fully resident in SBUF as bf16
    (64 KiB/partition each), streamed in fine f-slices at kernel start
    so compute begins after the first ~1 MiB lands.
  - Per 512-token chunk, two phases:
        phase 1:  hT[f,:] = gelu_tanh(sum_k W1[k,f].T @ xT[k,:])  all 32
                  f-tiles, kept resident in SBUF (bf16)
        phase 2:  yT[hs,:] += W2[f,hs].T @ hT[f,:]  accumulated over all
                  32 f-tiles directly in 8 PSUM banks (fp32) -> single
                  bf16 store per h-subtile, no partial-sum DRAM traffic.
  - Host applies combine weights + b2 and scatter-adds back ("combine").

  Matmuls run in bf16 (full-rate, FWL weight loads); PSUM accumulation
  is fp32.  rel-err vs the fp32 reference ~3e-3, well inside 2e-2.

The kernel is compiled once per (C_pad, chunk-structure, biases-zero)
configuration and cached in-process.
"""

import os
import sys
import numpy as np

for _p in ("/opt/trn_rl_repo", "/root/.axon_site/_ro/trn_rl_repo"):
    if _p not in sys.path and os.path.isdir(_p):
        sys.path.append(_p)

import concourse.bacc as bacc  # noqa: E402
import concourse.tile as tile  # noqa: E402
from concourse import mybir  # noqa: E402
from concourse.bass_utils import run_bass_kernel_spmd  # noqa: E402

# Problem shapes (hardcoded per spec)
B, S, H, F, E = 4, 2048, 1024, 4096, 8
T = B * S
TOP_K = 2
N_CORES = 8
P = 128
KH = H // P          # 8   H-contraction subtiles
FT = F // P          # 32  f-tiles
HS = H // P          # 8   output H subtiles
CH = 512             # token chunk (phase-1/2 granularity)
WG = 4               # f-tiles per weight-stream DMA slice

F32 = mybir.dt.float32
BF16 = mybir.dt.bfloat16

_CACHE: dict = {}
LAST_RESULT = None  # BassKernelResults of the most recent run (for test.py)


def _chunks_for(c_pad: int) -> tuple:
    out = [CH] * (c_pad // CH)
    if c_pad % CH:
        out.append(c_pad % CH)
    return tuple(out)


def _build(c_pad: int, chunks: tuple, use_b1: bool, mm_dt):
    nc = bacc.Bacc(
        "TRN2",
        target_bir_lowering=False,
        debug=False,
        enable_asserts=False,
        num_devices=N_CORES,
    )

    nch = len(chunks)
    # x packed chunk-contiguous: xd[p, ci, k, t] = x.T[k*128+p, ci*CH+t]
    xd = nc.dram_tensor("xd", [P, nch, KH, CH], mm_dt, kind="ExternalInput").ap()
    # w1d[p, f, k, q] = W1[k*128+p, f*128+q]
    w1d = nc.dram_tensor("w1d", [P, FT, KH, P], mm_dt, kind="ExternalInput").ap()
    # w2d[p, f, h]   = W2[f*128+p, h]
    w2d = nc.dram_tensor("w2d", [P, FT, H], mm_dt, kind="ExternalInput").ap()
    if use_b1:
        b1d = nc.dram_tensor("b1d", [P, FT], F32, kind="ExternalInput").ap()
    # yd[p, hs, t] = y[t, hs*128+p]  (bf16; host upcasts + combines)
    yd = nc.dram_tensor("yd", [P, HS, c_pad], BF16, kind="ExternalOutput").ap()

    gelu = mybir.ActivationFunctionType.Gelu_apprx_tanh

    with tile.TileContext(nc) as tc:
        with (
            tc.tile_pool(name="w1p", bufs=FT // WG) as w1p,
            tc.tile_pool(name="w2p", bufs=FT // WG) as w2p,
            tc.tile_pool(name="xp", bufs=2) as xp,
            tc.tile_pool(name="hp", bufs=FT + 1) as hp,
            tc.tile_pool(name="yp", bufs=10) as yp,
            tc.tile_pool(name="bp", bufs=1) as bp,
            tc.tile_pool(name="pp", bufs=1, space="PSUM") as pp,
        ):
            # All DMA issue goes on the sync (SP) ring: DMA_DIRECT2D issue
            # instructions cost ~0.6-1.3us each on the issuing engine's
            # queue, and on the scalar ring they delay the gelu ACT-table
            # load behind the whole weight stream (measured 33us PE stall).
            if use_b1:
                b1t = bp.tile([P, FT], F32)
                nc.sync.dma_start(b1t[:], b1d[:])

            # first x chunk ahead of the weight stream (needed immediately;
            # halves so GEMM1 k=0..3 can start on the first 0.5 MiB)
            xts: dict = {}
            xts[0] = xp.tile([P, KH, CH], mm_dt, tag="xt", name="xt_0")
            nc.sync.dma_start(xts[0][:, :4, : chunks[0]], xd[:, 0, :4, : chunks[0]])
            nc.sync.dma_start(xts[0][:, 4:, : chunks[0]], xd[:, 0, 4:, : chunks[0]])

            # resident weights, streamed in WG-f-tile slices (w1/w2
            # interleaved so phase 2 of chunk 0 never waits long)
            w1g = []
            w2g = []
            for g in range(FT // WG):
                t1 = w1p.tile([P, WG, KH, P], mm_dt, tag="w1g", name=f"w1g_{g}")
                nc.sync.dma_start(t1[:], w1d[:, g * WG : (g + 1) * WG])
                w1g.append(t1)
                t2 = w2p.tile([P, WG, H], mm_dt, tag="w2g", name=f"w2g_{g}")
                nc.sync.dma_start(t2[:], w2d[:, g * WG : (g + 1) * WG])
                w2g.append(t2)

            coff = 0
            for ci, nt in enumerate(chunks):
                xt = xts.pop(ci)
                # prefetch next chunk's tokens (behind the weight stream)
                if ci + 1 < nch:
                    nxt = chunks[ci + 1]
                    xts[ci + 1] = xp.tile(
                        [P, KH, CH], mm_dt, tag="xt", name=f"xt_{ci + 1}"
                    )
                    nc.sync.dma_start(
                        xts[ci + 1][:, :, :nxt], xd[:, ci + 1, :, :nxt]
                    )

                # phase 1: hT = gelu(W1.T @ xT) for all 32 f-tiles
                hqs = []
                for f in range(FT):
                    g, j = divmod(f, WG)
                    pt = pp.tile([P, CH], F32, tag="pt", bufs=2)
                    for k in range(KH):
                        nc.tensor.matmul(
                            pt[:, :nt],
                            w1g[g][:, j, k, :],
                            xt[:, k, :nt],
                            start=(k == 0),
                            stop=(k == KH - 1),
                        )
                    hq = hp.tile([P, CH], mm_dt, tag="hq", name=f"hq_{ci}_{f}")
                    bias = b1t[:, f : f + 1] if use_b1 else 0.0
                    nc.scalar.activation(hq[:, :nt], pt[:, :nt], gelu, bias=bias)
                    hqs.append(hq)

                # phase 2: yT[hs] = sum_f W2[f,hs].T @ hT[f], accumulated in
                # PSUM over all 32 f-tiles.  Two passes over the h-subtiles
                # (0-5, then 6-7) keep peak PSUM usage at 6 accumulator banks
                # + 2 phase-1 banks; the second pass re-reads the SBUF-resident
                # hqs, costing nothing extra on the PE.
                def gemm2_pass(hs_list):
                    pys = [
                        pp.tile([P, CH], F32, tag="py", bufs=6,
                                name=f"py_{ci}_{hs}")
                        for hs in hs_list
                    ]
                    for f in range(FT):
                        g, j = divmod(f, WG)
                        for pi, hs in enumerate(hs_list):
                            nc.tensor.matmul(
                                pys[pi][:, :nt],
                                w2g[g][:, j, hs * P : (hs + 1) * P],
                                hqs[f][:, :nt],
                                start=(f == 0),
                                stop=(f == FT - 1),
                            )
                    # evacuate: PSUM fp32 -> SBUF bf16 -> DRAM
                    # (DVE even, ACT odd, in PE-demand order)
                    for pi, hs in enumerate(hs_list):
                        yt = yp.tile([P, CH], BF16, tag="yt")
                        if pi % 2 == 0:
                            nc.vector.tensor_copy(yt[:, :nt], pys[pi][:, :nt])
                        else:
                            nc.scalar.copy(yt[:, :nt], pys[pi][:, :nt])
                        nc.sync.dma_start(
                            yd[:, hs, coff : coff + nt], yt[:, :nt]
                        )

                gemm2_pass(list(range(6)))
                gemm2_pass([6, 7])
                coff += nt

    nc.compile()
    return nc


def _route(x2d, Wg):
    """Replicates reference router: softmax -> top-2 -> renormalize."""
    logits = x2d @ Wg  # [T, E] fp32
    m = logits.max(axis=-1, keepdims=True)
    p = np.exp(logits - m, dtype=np.float32)
    p /= p.sum(axis=-1, keepdims=True)
    # jax.lax.top_k: values descending, ties broken by lower index.
    order = np.argsort(-p, axis=-1, kind="stable")
    top_i = order[:, :TOP_K]  # [T, 2]
    top_p = np.take_along_axis(p, top_i, axis=-1)
    top_p = top_p / top_p.sum(axis=-1, keepdims=True)
    return top_i, top_p


def kernel(x, Wg, W1, b1, W2, b2):
    global LAST_RESULT
    x = np.ascontiguousarray(np.asarray(x, dtype=np.float32))
    Wg = np.ascontiguousarray(np.asarray(Wg, dtype=np.float32))
    W1 = np.ascontiguousarray(np.asarray(W1, dtype=np.float32))
    b1 = np.ascontiguousarray(np.asarray(b1, dtype=np.float32))
    W2 = np.ascontiguousarray(np.asarray(W2, dtype=np.float32))
    b2 = np.ascontiguousarray(np.asarray(b2, dtype=np.float32))

    x2d = x.reshape(T, H)
    top_i, top_p = _route(x2d, Wg)

    rows = [None] * E
    gval = [None] * E
    for e in range(E):
        r, slot = np.nonzero(top_i == e)
        rows[e] = r
        gval[e] = top_p[r, slot]

    c_pad = max(CH, max(len(r) for r in rows))
    chunks = _chunks_for(c_pad)
    use_b1 = bool(np.any(b1))

    mm_dt = {
        "bf16": BF16,
        "fp32r": mybir.dt.float32r,
        "fp32": F32,
    }[os.environ.get("KERNEL_MMDT", "bf16")]
    key = (c_pad, chunks, use_b1, str(mm_dt))
    if key not in _CACHE:
        _CACHE[key] = _build(c_pad, chunks, use_b1, mm_dt)
    nc = _CACHE[key]

    np_dt = mybir.dt.np(mm_dt)
    nch = len(chunks)
    in_maps = []
    for e in range(E):
        ce = len(rows[e])
        xt = np.zeros((H, c_pad), np.float32)
        xt[:, :ce] = x2d[rows[e]].T
        # [H, c_pad] -> [P, nch, KH, CH] chunk-contiguous
        xpk = np.zeros((P, nch, KH, CH), np_dt)
        xk = xt.reshape(KH, P, c_pad)
        coff = 0
        for ci, nt in enumerate(chunks):
            xpk[:, ci, :, :nt] = (
                xk[:, :, coff : coff + nt].transpose(1, 0, 2).astype(np_dt)
            )
            coff += nt
        m = {
            "xd": np.ascontiguousarray(xpk),
            "w1d": np.ascontiguousarray(
                W1[e].reshape(KH, P, FT, P).transpose(1, 2, 0, 3).astype(np_dt)
            ),
            "w2d": np.ascontiguousarray(
                W2[e].reshape(FT, P, H).transpose(1, 0, 2).astype(np_dt)
            ),
        }
        if use_b1:
            m["b1d"] = np.ascontiguousarray(b1[e].reshape(FT, P).T)
        in_maps.append(m)

    trace = os.environ.get("KERNEL_TRACE", "") == "1"
    res = run_bass_kernel_spmd(
        nc,
        in_maps,
        core_ids=list(range(N_CORES)),
        trace=trace,
        trace_cores=[0] if trace else None,
    )
    LAST_RESULT = res

    out = np.zeros((T, H), np.float32)
    for e in range(E):
        ce = len(rows[e])
        yt = res.results[e]["yd"]  # [P, HS, c_pad] bf16
        y = yt.transpose(2, 1, 0).reshape(c_pad, H)[:ce].astype(np.float32)
        out[rows[e]] += gval[e][:, None] * (y + b2[e][None, :])

    return out.reshape(B, S, H)


# revision 11
# speedup vs baseline: 1.2227x; 1.0657x over previous
"""MoE FFN (top-2 of 8 experts) Trainium2 kernel.

Strategy (expert-parallel across 8 NeuronCores):
  - Host computes the (tiny) router: logits = x@Wg, softmax, top-2,
    renormalized combine weights.  Tokens are gathered per expert on the
    host ("all-to-all dispatch" done at sharding time), transposed to
    [H, C] so both FFN GEMMs run with natural weight layouts on device.
  - Core e runs the FFN for expert e over its C_pad gathered tokens.
    Both expert weight matrices live fully resident in SBUF as bf16
    (64 KiB/partition each), streamed in fine f-slices at kernel start
    so compute begins after the first ~1 MiB lands.
  - Per 512-token chunk, two phases:
        phase 1:  hT[f,:] = gelu_tanh(sum_k W1[k,f].T @ xT[k,:])  all 32
                  f-tiles, kept resident in SBUF (bf16)
        phase 2:  yT[hs,:] += W2[f,hs].T @ hT[f,:]  accumulated over all
                  32 f-tiles directly in 8 PSUM banks (fp32) -> single
                  bf16 store per h-subtile, no partial-sum DRAM traffic.
  - Host applies combine weights + b2 and scatter-adds back ("combine").

  Matmuls run in bf16 (full-rate, FWL weight loads); PSUM accumulation
  is fp32.  rel-err vs the fp32 reference ~3e-3, well inside 2e-2.

The kernel is compiled once per (C_pad, chunk-structure, biases-zero)
configuration and cached in-process.
"""

import os
import sys
import numpy as np

for _p in ("/opt/trn_rl_repo", "/root/.axon_site/_ro/trn_rl_repo"):
    if _p not in sys.path and os.path.isdir(_p):
        sys.path.append(_p)

import concourse.bacc as bacc  # noqa: E402
import concourse.tile as tile  # noqa: E402
from concourse import mybir  # noqa: E402
from concourse.bass_utils import run_bass_kernel_spmd  # noqa: E402

# Problem shapes (hardcoded per spec)
B, S, H, F, E = 4, 2048, 1024, 4096, 8
T = B * S
TOP_K = 2
N_CORES = 8
P = 128
KH = H // P          # 8   H-contraction subtiles
FT = F // P          # 32  f-tiles
HS = H // P          # 8   output H subtiles
CH = 512             # token chunk (phase-1/2 granularity)
WG = 4               # f-tiles per weight-stream DMA slice

F32 = mybir.dt.float32
BF16 = mybir.dt.bfloat16

_CACHE: dict = {}
LAST_RESULT = None  # BassKernelResults of the most recent run (for test.py)


def _chunks_for(c_pad: int) -> tuple:
    out = [CH] * (c_pad // CH)
    if c_pad % CH:
        out.append(c_pad % CH)
    return tuple(out)


def _build(c_pad: int, chunks: tuple, use_b1: bool, mm_dt):
    nc = bacc.Bacc(
        "TRN2",
        target_bir_lowering=False,
        debug=False,
        enable_asserts=False,
        num_devices=N_CORES,
    )

    nch = len(chunks)
    # x packed chunk-contiguous: xd[p, ci, k, t] = x.T[k*128+p, ci*CH+t]
    xd = nc.dram_tensor("xd", [P, nch, KH, CH], mm_dt, kind="ExternalInput").ap()
    # w1d[p, f, k, q] = W1[k*128+p, f*128+q]
    w1d = nc.dram_tensor("w1d", [P, FT, KH, P], mm_dt, kind="ExternalInput").ap()
    # w2d[p, f, h]   = W2[f*128+p, h]
    w2d = nc.dram_tensor("w2d", [P, FT, H], mm_dt, kind="ExternalInput").ap()
    if use_b1:
        b1d = nc.dram_tensor("b1d", [P, FT], F32, kind="ExternalInput").ap()
    # yd[p, hs, t] = y[t, hs*128+p]  (bf16; host upcasts + combines)
    yd = nc.dram_tensor("yd", [P, HS, c_pad], BF16, kind="ExternalOutput").ap()

    gelu = mybir.ActivationFunctionType.Gelu_apprx_tanh

    with tile.TileContext(nc) as tc:
        with (
            tc.tile_pool(name="w1p", bufs=1) as w1p,
            tc.tile_pool(name="w2p", bufs=1) as w2p,
            tc.tile_pool(name="xp", bufs=2) as xp,
            tc.tile_pool(name="hp", bufs=FT + 1) as hp,
            tc.tile_pool(name="yp", bufs=10) as yp,
            tc.tile_pool(name="bp", bufs=1) as bp,
            tc.tile_pool(name="pp", bufs=1, space="PSUM") as pp,
        ):
            # Warm-up: the PE clock sits at 1.2 GHz until ~3.4us of sustained
            # activity (HAM gate).  Run dummy matmuls on uninitialized SBUF
            # scratch during the initial DMA wait so real matmuls start at
            # 2.4 GHz.  Results land in a pt-pool slot and are never read.
            wsrc = bp.tile([P, CH], mm_dt, name="warm_src")
            nc.vector.memset(wsrc[:, :P], 0.0)
            wdst = pp.tile([P, CH], F32, tag="pt", bufs=2, name="warm_dst")
            for _ in range(48):
                nc.tensor.matmul(
                    wdst[:, :P], wsrc[:, :P], wsrc[:, :P], start=True, stop=True
                )

            # All DMA issue goes on the sync (SP) ring: DMA_DIRECT2D issue
            # instructions cost ~0.6-1.3us each on the issuing engine's
            # queue, and on the scalar ring they delay the gelu ACT-table
            # load behind the whole weight stream (measured 33us PE stall).
            if use_b1:
                b1t = bp.tile([P, FT], F32)
                nc.sync.dma_start(b1t[:], b1d[:])

            # first x chunk ahead of the weight stream (needed immediately;
            # halves so GEMM1 k=0..3 can start on the first 0.5 MiB)
            xts: dict = {}
            xts[0] = xp.tile([P, KH, CH], mm_dt, tag="xt", name="xt_0")
            nc.sync.dma_start(xts[0][:, :4, : chunks[0]], xd[:, 0, :4, : chunks[0]])

            # resident weights, streamed in slices; the first two w1 slices
            # are fine (f0 alone, then f1-3) so GEMM1 starts after ~0.75 MiB
            w1sl = [(0, 1), (1, 4)] + [(g, g + WG) for g in range(WG, FT, WG)]
            w2sl = [(g, g + WG) for g in range(0, FT, WG)]
            w1map: list = [None] * FT
            w2map: list = [None] * FT

            def _w1_load(si):
                lo, hi = w1sl[si]
                t = w1p.tile([P, hi - lo, KH, P], mm_dt, tag=f"w1g{si}",
                             name=f"w1g_{si}", bufs=1)
                nc.sync.dma_start(t[:], w1d[:, lo:hi])
                for f in range(lo, hi):
                    w1map[f] = (t, f - lo)

            def _w2_load(si):
                lo, hi = w2sl[si]
                t = w2p.tile([P, hi - lo, H], mm_dt, tag=f"w2g{si}",
                             name=f"w2g_{si}", bufs=1)
                nc.sync.dma_start(t[:], w2d[:, lo:hi])
                for f in range(lo, hi):
                    w2map[f] = (t, f - lo)

            _w1_load(0)
            _w1_load(1)
            nc.sync.dma_start(xts[0][:, 4:, : chunks[0]], xd[:, 0, 4:, : chunks[0]])
            for si in range(2, len(w1sl)):
                _w1_load(si)
                _w2_load(si - 2)
            _w2_load(len(w2sl) - 2)
            _w2_load(len(w2sl) - 1)

            coff = 0
            for ci, nt in enumerate(chunks):
                xt = xts.pop(ci)
                # prefetch next chunk's tokens (behind the weight stream)
                if ci + 1 < nch:
                    nxt = chunks[ci + 1]
                    xts[ci + 1] = xp.tile(
                        [P, KH, CH], mm_dt, tag="xt", name=f"xt_{ci + 1}"
                    )
                    nc.sync.dma_start(
                        xts[ci + 1][:, :, :nxt], xd[:, ci + 1, :, :nxt]
                    )

                # phase 1: hT = gelu(W1.T @ xT) for all 32 f-tiles
                hqs = []
                for f in range(FT):
                    w1t, j = w1map[f]
                    pt = pp.tile([P, CH], F32, tag="pt", bufs=2)
                    for k in range(KH):
                        nc.tensor.matmul(
                            pt[:, :nt],
                            w1t[:, j, k, :],
                            xt[:, k, :nt],
                            start=(k == 0),
                            stop=(k == KH - 1),
                        )
                    hq = hp.tile([P, CH], mm_dt, tag="hq", name=f"hq_{ci}_{f}")
                    bias = b1t[:, f : f + 1] if use_b1 else 0.0
                    nc.scalar.activation(hq[:, :nt], pt[:, :nt], gelu, bias=bias)
                    hqs.append(hq)

                # phase 2: yT[hs] = sum_f W2[f,hs].T @ hT[f], accumulated in
                # PSUM over all 32 f-tiles.  Two passes over the h-subtiles
                # (0-5, then 6-7) keep peak PSUM usage at 6 accumulator banks
                # + 2 phase-1 banks; the second pass re-reads the SBUF-resident
                # hqs, costing nothing extra on the PE.
                def gemm2_pass(hs_list):
                    pys = [
                        pp.tile([P, CH], F32, tag="py", bufs=6,
                                name=f"py_{ci}_{hs}")
                        for hs in hs_list
                    ]
                    for f in range(FT):
                        w2t, j = w2map[f]
                        for pi, hs in enumerate(hs_list):
                            nc.tensor.matmul(
                                pys[pi][:, :nt],
                                w2t[:, j, hs * P : (hs + 1) * P],
                                hqs[f][:, :nt],
                                start=(f == 0),
                                stop=(f == FT - 1),
                            )
                    # evacuate: PSUM fp32 -> SBUF bf16 -> DRAM
                    # (DVE even, ACT odd, in PE-demand order)
                    for pi, hs in enumerate(hs_list):
                        yt = yp.tile([P, CH], BF16, tag="yt")
                        if pi % 2 == 0:
                            nc.vector.tensor_copy(yt[:, :nt], pys[pi][:, :nt])
                        else:
                            nc.scalar.copy(yt[:, :nt], pys[pi][:, :nt])
                        nc.sync.dma_start(
                            yd[:, hs, coff : coff + nt], yt[:, :nt]
                        )

                gemm2_pass(list(range(6)))
                gemm2_pass([6, 7])
                coff += nt

    nc.compile()
    return nc


def _route(x2d, Wg):
    """Replicates reference router: softmax -> top-2 -> renormalize."""
    logits = x2d @ Wg  # [T, E] fp32
    m = logits.max(axis=-1, keepdims=True)
    p = np.exp(logits - m, dtype=np.float32)
    p /= p.sum(axis=-1, keepdims=True)
    # jax.lax.top_k: values descending, ties broken by lower index.
    order = np.argsort(-p, axis=-1, kind="stable")
    top_i = order[:, :TOP_K]  # [T, 2]
    top_p = np.take_along_axis(p, top_i, axis=-1)
    top_p = top_p / top_p.sum(axis=-1, keepdims=True)
    return top_i, top_p


def kernel(x, Wg, W1, b1, W2, b2):
    global LAST_RESULT
    x = np.ascontiguousarray(np.asarray(x, dtype=np.float32))
    Wg = np.ascontiguousarray(np.asarray(Wg, dtype=np.float32))
    W1 = np.ascontiguousarray(np.asarray(W1, dtype=np.float32))
    b1 = np.ascontiguousarray(np.asarray(b1, dtype=np.float32))
    W2 = np.ascontiguousarray(np.asarray(W2, dtype=np.float32))
    b2 = np.ascontiguousarray(np.asarray(b2, dtype=np.float32))

    x2d = x.reshape(T, H)
    top_i, top_p = _route(x2d, Wg)

    rows = [None] * E
    gval = [None] * E
    for e in range(E):
        r, slot = np.nonzero(top_i == e)
        rows[e] = r
        gval[e] = top_p[r, slot]

    c_pad = max(CH, max(len(r) for r in rows))
    chunks = _chunks_for(c_pad)
    use_b1 = bool(np.any(b1))

    mm_dt = {
        "bf16": BF16,
        "fp32r": mybir.dt.float32r,
        "fp32": F32,
    }[os.environ.get("KERNEL_MMDT", "bf16")]
    key = (c_pad, chunks, use_b1, str(mm_dt))
    if key not in _CACHE:
        _CACHE[key] = _build(c_pad, chunks, use_b1, mm_dt)
    nc = _CACHE[key]

    np_dt = mybir.dt.np(mm_dt)
    nch = len(chunks)
    in_maps = []
    for e in range(E):
        ce = len(rows[e])
        xt = np.zeros((H, c_pad), np.float32)
        xt[:, :ce] = x2d[rows[e]].T
        # [H, c_pad] -> [P, nch, KH, CH] chunk-contiguous
        xpk = np.zeros((P, nch, KH, CH), np_dt)
        xk = xt.reshape(KH, P, c_pad)
        coff = 0
        for ci, nt in enumerate(chunks):
            xpk[:, ci, :, :nt] = (
                xk[:, :, coff : coff + nt].transpose(1, 0, 2).astype(np_dt)
            )
            coff += nt
        m = {
            "xd": np.ascontiguousarray(xpk),
            "w1d": np.ascontiguousarray(
                W1[e].reshape(KH, P, FT, P).transpose(1, 2, 0, 3).astype(np_dt)
            ),
            "w2d": np.ascontiguousarray(
                W2[e].reshape(FT, P, H).transpose(1, 0, 2).astype(np_dt)
            ),
        }
        if use_b1:
            m["b1d"] = np.ascontiguousarray(b1[e].reshape(FT, P).T)
        in_maps.append(m)

    trace = os.environ.get("KERNEL_TRACE", "") == "1"
    res = run_bass_kernel_spmd(
        nc,
        in_maps,
        core_ids=list(range(N_CORES)),
        trace=trace,
        trace_cores=[0] if trace else None,
    )
    LAST_RESULT = res

    out = np.zeros((T, H), np.float32)
    for e in range(E):
        ce = len(rows[e])
        yt = res.results[e]["yd"]  # [P, HS, c_pad] bf16
        y = yt.transpose(2, 1, 0).reshape(c_pad, H)[:ce].astype(np.float32)
        out[rows[e]] += gval[e][:, None] * (y + b2[e][None, :])

    return out.reshape(B, S, H)
